# revision 20
# baseline (speedup 1.0000x reference)
"""Ewald reciprocal-space kernel for Trainium2 (8 NeuronCores, SPMD).

Math (per batch b):
    s        = cell_inv @ x          (fractional coords)
    theta    = 2*pi * (kvec . s)     (B, N, NK) phases
    S_re/S_im= sum_n q_n {cos,sin}(theta)          (structure factor)
    recip_n  = sum_k expfac_k (S_re cos + S_im sin)
    phi      = recip * BOHR/(pi*V) - q * 2*bewald*BOHR/sqrt(pi)
    returns (0.5*q*phi, phi)

Sharding: 8 cores = 2 batches x 4 k-shards (1024 k-vectors each). Each core
computes its full-N, shard-K contribution to recip with no collectives; host
sums the 4 shard partials per batch and applies the final affine.

Device pipeline per core (N=4096 as 32 chunks of 128 partitions):
  C = -r in PSUM via 4 accumulating matmuls: u, +M, -M, -u (M = 1.5*2^23
      magic; -u via host-negated kvec rows (kmN); each psum accumulate
      rounds in fp32 so C = round(u) - u exactly, |C| <= 1/2 - verified
      bit-exact on HW).
  sin half: ACT Sin reads C from PSUM directly (scale -2pi), fp16 out.
  cos half: DVE add_range_wrap(C - 1/4) -> fp16 phases, Sin per 2-chunk batch.
  S: flipped matmuls (lhsT = 128-col cs slabs, rhs = q chunk -> [128,1] out,
     PE cost ~ output free size) -> per-chunk psum, DVE-accumulated in SBUF.
  cs chunks DMA-transposed (xbar) into csT - SP queue only (ACT-queue
     transposes corrupt data on real HW); last batch transposes split into
     sin/cos halves to shorten the tail.
  recip: flipped matmuls, 16-deep psum chains per n-chunk -> [128, 32],
     bulk of the store overlapped with the final chains.
  PE p-state warmed with dummy rank-1 matmuls during input loads.
"""

import math
from contextlib import ExitStack

import numpy as np

BOHR = 1.8897261258369282

B, N, NK = 2, 4096, 4096
NCORES = 8
KSH = NK // 4          # k-vectors per core
NCH = N // 128         # 32 n-chunks
NSL = 2 * KSH // 128   # 16 slices: 0-7 sin(k=128s+p), 8-15 cos
QB = 2                 # chunks per cos-Sin batch

_PROG = {}


def _build_program():
    import concourse.bass as bass
    import concourse.bacc as bacc
    import concourse.tile as tile
    import concourse.mybir as mybir

    F32 = mybir.dt.float32
    F32R = mybir.dt.float32r
    F16 = mybir.dt.float16
    MAGIC = 12582912.0          # 1.5 * 2**23: fp32 round-to-nearest-integer
    # two fp32 ulps below 2*pi so |scale * r| <= pi holds for r = +-1/2 exactly
    NEG2PI = -6.28318452835083

    nc = bacc.Bacc(trn_type="TRN2", target_bir_lowering=False, debug=False)

    coordsT_d = nc.dram_tensor("coordsT", [3, N], F32, kind="ExternalInput").ap()
    qT_d = nc.dram_tensor("qT", [128, NCH], F32, kind="ExternalInput").ap()
    km2_d = nc.dram_tensor("km2", [3, 2 * KSH], F32, kind="ExternalInput").ap()
    ef2_d = nc.dram_tensor("ef2", [128, NSL], F32, kind="ExternalInput").ap()
    recp_d = nc.dram_tensor("recp", [128, NCH], F32, kind="ExternalOutput").ap()

    with tile.TileContext(nc) as tc, ExitStack() as ctx:
        const = ctx.enter_context(tc.tile_pool(name="const", bufs=1))
        pu = ctx.enter_context(tc.tile_pool(name="pu", bufs=2, space="PSUM"))
        wk = ctx.enter_context(tc.tile_pool(name="wk", bufs=4))

        # ---- loads: split the PE-critical ones across both queues ----
        km2 = const.tile([3, 2 * KSH], F32R)
        nc.sync.dma_start(out=km2[:, 0:KSH], in_=km2_d[:, 0:KSH].bitcast(F32R))
        nc.scalar.dma_start(out=km2[:, KSH:2 * KSH],
                            in_=km2_d[:, KSH:2 * KSH].bitcast(F32R))
        km = km2[:, 0:KSH]
        kmn = km2[:, KSH:2 * KSH]
        cts = const.tile([3, N], F32R)
        nc.sync.dma_start(out=cts[:, 0:1024], in_=coordsT_d[:, 0:1024].bitcast(F32R))
        nc.sync.dma_start(out=cts[:, 1024:N],
                          in_=coordsT_d[:, 1024:N].bitcast(F32R))
        qt = const.tile([128, NCH], F32)
        nc.sync.dma_start(out=qt[:, :], in_=qT_d)
        ef = const.tile([128, NSL], F32)
        nc.sync.dma_start(out=ef[:, :], in_=ef2_d)

        qt16 = const.tile([128, NCH], F16)
        nc.vector.tensor_copy(qt16[:, :], qt[:, :])
        ones = const.tile([1, 128], F32R)
        nc.vector.memset(ones.bitcast(F32)[:, :], 1.0)
        mrow = const.tile([1, 512], F32R)
        nc.vector.memset(mrow.bitcast(F32)[:, :], MAGIC)
        mrow_n = const.tile([1, 512], F32R)
        nc.vector.memset(mrow_n.bitcast(F32)[:, :], -MAGIC)
        s_run = const.tile([128, NSL], F32)
        nc.vector.memset(s_run[:, :], 0.0)
        sin_warm = const.tile([128, NSL], F16)
        nc.scalar.activation(sin_warm[:, :], s_run[:, :],
                             mybir.ActivationFunctionType.Sin,
                             bias=0.0, scale=NEG2PI)

        # warm the PE p-state during input loads: dummy rank-1 matmuls with
        # no input deps beyond the tiny memset consts
        warm = pu.tile([128, 2 * KSH], F32, tag="C", name="warm")
        for _ in range(10):
            nc.tensor.matmul(warm[0:1, 0:512], lhsT=ones[:, 0:1],
                             rhs=mrow[:, :], start=True, stop=True)

        csT = const.tile([128, NSL, N], F16)

        # Unified psum tile per chunk: cols [0:1024] = -r (PE 4-pass magic),
        # cols [1024:2048] = -r_c (DVE wrap, written back into PSUM), so ONE
        # Sin covers both halves. The small S accumulator aliases into the
        # consumed region of the same tile (write-after-read ordered by Tile).
        C_of = {}

        def emit_front(t):
            C = pu.tile([128, 2 * KSH], F32, tag="C", name=f"C{t}")
            C_of[t] = C
            for off in range(0, KSH, 512):
                co = C[:, off:off + 512]
                nc.tensor.matmul(co, lhsT=cts[:, 128 * t:128 * (t + 1)],
                                 rhs=km[:, off:off + 512], start=True, stop=False)
                nc.tensor.matmul(co, lhsT=ones[:, :], rhs=mrow[:, :],
                                 start=False, stop=False)
                nc.tensor.matmul(co, lhsT=ones[:, :], rhs=mrow_n[:, :],
                                 start=False, stop=False)
                nc.tensor.matmul(co, lhsT=cts[:, 128 * t:128 * (t + 1)],
                                 rhs=kmn[:, off:off + 512], start=False, stop=True)
            nc.vector.add_range_wrap(out=C[:, KSH:2 * KSH], in_=C[:, 0:KSH],
                                     shift=-0.25, bound=0.5, period=1.0)
            cs = wk.tile([128, 2 * KSH], F16, tag="cs", name=f"cs{t}")
            nc.scalar.activation(cs[:, :], C[:, :],
                                 mybir.ActivationFunctionType.Sin,
                                 bias=0.0, scale=NEG2PI)
            return cs

        def emit_back(t, cs):
            """S matmuls (into the dead C region) + transpose."""
            s_ch = C_of[t][:, 0:NSL]
            for s in range(NSL):
                nc.tensor.matmul(s_ch[:, s:s + 1],
                                 lhsT=cs[:, 128 * s:128 * (s + 1)],
                                 rhs=qt16[:, t:t + 1], start=True, stop=True)
            nc.vector.tensor_tensor(out=s_run[:, :], in0=s_run[:, :],
                                    in1=s_ch[:, :], op=mybir.AluOpType.add)
            nc.sync.dma_start_transpose(
                out=csT[:, :, 128 * t:128 * (t + 1)], in_=cs)

        prev = None
        for t in range(NCH):
            cs = emit_front(t)
            if prev is not None:
                emit_back(prev[0], prev[1])
            prev = (t, cs)
        emit_back(prev[0], prev[1])

        # ---- w = expfac * S (fp16 columns) ----
        w = const.tile([128, NSL], F16)
        nc.vector.tensor_tensor(out=w[:, :], in0=s_run[:, :], in1=ef[:, :],
                                op=mybir.AluOpType.mult)

        # ---- pass 2: recip[128c+p] via 16-deep flipped-matmul psum chains.
        # r_acc aliases into the final C tile (bank 1, long dead region).
        # Store in two parts so the bulk ships while the last chunks (whose
        # transposes land last) finish their chains. ----
        r_acc = C_of[NCH - 1][:, 512:512 + NCH]
        outsb = const.tile([128, NCH], F32)
        SPLIT = NCH - 4
        for c in range(NCH):
            for s in range(NSL):
                nc.tensor.matmul(r_acc[:, c:c + 1],
                                 lhsT=csT[:, s, 128 * c:128 * (c + 1)],
                                 rhs=w[:, s:s + 1],
                                 start=(s == 0), stop=(s == NSL - 1))
            if c == SPLIT - 1:
                nc.vector.tensor_copy(outsb[:, 0:SPLIT], r_acc[:, 0:SPLIT])
                nc.sync.dma_start(out=recp_d[:, 0:SPLIT],
                                  in_=outsb[:, 0:SPLIT])
        nc.vector.tensor_copy(outsb[:, SPLIT:NCH], r_acc[:, SPLIT:NCH])
        nc.sync.dma_start(out=recp_d[:, SPLIT:NCH], in_=outsb[:, SPLIT:NCH])

    nc.compile()
    return nc


def _get_prog():
    if "prog" not in _PROG:
        _PROG["prog"] = _build_program()
    return _PROG["prog"]


def _make_in_maps(coords, q, cell_inv, kvec, expfac):
    in_maps = []
    for c in range(NCORES):
        b, ks = divmod(c, NCORES // B)
        sl = slice(KSH * ks, KSH * (ks + 1))
        ct = np.ascontiguousarray(coords[b].T, dtype=np.float32)
        ef = np.ascontiguousarray(expfac[sl], dtype=np.float32)
        ef8 = ef.reshape(NSL // 2, 128).T          # [128, 8]
        kmT = np.ascontiguousarray(
            (kvec[sl].astype(np.float32) @ cell_inv.astype(np.float32)).T)
        in_maps.append({
            "coordsT": ct,
            "qT": np.ascontiguousarray(q[b].reshape(NCH, 128).T, dtype=np.float32),
            "km2": np.ascontiguousarray(np.concatenate([kmT, -kmT], axis=1)),
            "ef2": np.ascontiguousarray(np.concatenate([ef8, ef8], axis=1)),
        })
    return in_maps


def _finalize(results, q, volume, bewald):
    recip = np.zeros((B, N), np.float32)
    for c in range(NCORES):
        b = c // (NCORES // B)
        recip[b] += results[c]["recp"].T.reshape(-1)
    scale1 = np.float32(BOHR / (math.pi * float(volume[0])))
    scale2 = np.float32(2.0 * float(bewald[0]) * BOHR / math.sqrt(math.pi))
    phi = (recip * scale1 - q.astype(np.float32) * scale2).astype(np.float32)
    e = (np.float32(0.5) * q.astype(np.float32) * phi).astype(np.float32)
    return e, phi


def kernel(coords, q, cell_inv, kvec, expfac, volume, bewald):
    from concourse.bass_utils import run_bass_kernel_spmd

    nc = _get_prog()
    in_maps = _make_in_maps(coords, q, cell_inv, kvec, expfac)
    res = run_bass_kernel_spmd(nc, in_maps, list(range(NCORES))).results
    return _finalize(res, q, volume, bewald)


# revision 21
# speedup vs baseline: 1.4474x; 1.4474x over previous
"""Ewald reciprocal-space kernel for Trainium2 (8 NeuronCores, SPMD).

Math (per batch b):
    s        = cell_inv @ x          (fractional coords)
    theta    = 2*pi * (kvec . s)     (B, N, NK) phases
    S_re/S_im= sum_n q_n {cos,sin}(theta)          (structure factor)
    recip_n  = sum_k expfac_k (S_re cos + S_im sin)
    phi      = recip * BOHR/(pi*V) - q * 2*bewald*BOHR/sqrt(pi)
    returns (0.5*q*phi, phi)

Sharding: 8 cores = 2 batches x 4 k-shards (1024 k-vectors each). Each core
computes its full-N, shard-K contribution to recip with no collectives; host
sums the 4 shard partials per batch and applies the final affine.

Device pipeline per core (N=4096 as 32 chunks of 128 partitions):
  C = -r in PSUM via 4 accumulating matmuls: u, +M, -M, -u (M = 1.5*2^23
      magic; -u via host-negated kvec rows (kmN); each psum accumulate
      rounds in fp32 so C = round(u) - u exactly, |C| <= 1/2 - verified
      bit-exact on HW).
  sin half: ACT Sin reads C from PSUM directly (scale -2pi), fp16 out.
  cos half: DVE add_range_wrap(C - 1/4) -> fp16 phases, Sin per 2-chunk batch.
  S: flipped matmuls (lhsT = 128-col cs slabs, rhs = q chunk -> [128,1] out,
     PE cost ~ output free size) -> per-chunk psum, DVE-accumulated in SBUF.
  cs chunks DMA-transposed (xbar) into csT - SP queue only (ACT-queue
     transposes corrupt data on real HW); last batch transposes split into
     sin/cos halves to shorten the tail.
  recip: flipped matmuls, 16-deep psum chains per n-chunk -> [128, 32],
     bulk of the store overlapped with the final chains.
  PE p-state warmed with dummy rank-1 matmuls during input loads.
"""

import math
from contextlib import ExitStack

import numpy as np

BOHR = 1.8897261258369282

B, N, NK = 2, 4096, 4096
NCORES = 8
KSH = NK // 4          # k-vectors per core
NCH = N // 128         # 32 n-chunks
NSL = 2 * KSH // 128   # 16 slices: 0-7 sin(k=128s+p), 8-15 cos
QB = 2                 # chunks per cos-Sin batch

_PROG = {}


def _build_program():
    import concourse.bass as bass
    import concourse.bacc as bacc
    import concourse.tile as tile
    import concourse.mybir as mybir

    F32 = mybir.dt.float32
    F32R = mybir.dt.float32r
    F16 = mybir.dt.float16
    MAGIC = 12582912.0          # 1.5 * 2**23: fp32 round-to-nearest-integer
    # two fp32 ulps below 2*pi so |scale * r| <= pi holds for r = +-1/2 exactly
    NEG2PI = -6.28318452835083

    nc = bacc.Bacc(trn_type="TRN2", target_bir_lowering=False, debug=False)

    coordsT_d = nc.dram_tensor("coordsT", [3, N], F32, kind="ExternalInput").ap()
    qT_d = nc.dram_tensor("qT", [128, NCH], F32, kind="ExternalInput").ap()
    km2_d = nc.dram_tensor("km2", [3, 2 * KSH], F32, kind="ExternalInput").ap()
    ef2_d = nc.dram_tensor("ef2", [128, NSL], F32, kind="ExternalInput").ap()
    recp_d = nc.dram_tensor("recp", [128, NCH], F32, kind="ExternalOutput").ap()

    with tile.TileContext(nc) as tc, ExitStack() as ctx:
        const = ctx.enter_context(tc.tile_pool(name="const", bufs=1))
        pu = ctx.enter_context(tc.tile_pool(name="pu", bufs=3, space="PSUM"))
        psm = ctx.enter_context(tc.tile_pool(name="psm", bufs=1, space="PSUM"))
        pacc = ctx.enter_context(tc.tile_pool(name="pacc", bufs=1, space="PSUM"))
        wk = ctx.enter_context(tc.tile_pool(name="wk", bufs=3))

        # ---- loads: C(0) needs cts head + km first (parallel queues),
        # kmn only at its 4th pass, the rest later ----
        cts = const.tile([3, N], F32R)
        nc.sync.dma_start(out=cts[:, 0:1024], in_=coordsT_d[:, 0:1024].bitcast(F32R))
        km2 = const.tile([3, 2 * KSH], F32R)
        nc.scalar.dma_start(out=km2[:, 0:KSH], in_=km2_d[:, 0:KSH].bitcast(F32R))
        nc.scalar.dma_start(out=km2[:, KSH:2 * KSH],
                            in_=km2_d[:, KSH:2 * KSH].bitcast(F32R))
        km = km2[:, 0:KSH]
        kmn = km2[:, KSH:2 * KSH]
        nc.sync.dma_start(out=cts[:, 1024:N],
                          in_=coordsT_d[:, 1024:N].bitcast(F32R))
        qt = const.tile([128, NCH], F32)
        nc.sync.dma_start(out=qt[:, :], in_=qT_d)
        ef = const.tile([128, NSL], F32)
        nc.sync.dma_start(out=ef[:, :], in_=ef2_d)

        qt16 = const.tile([128, NCH], F16)
        nc.vector.tensor_copy(qt16[:, :], qt[:, :])
        ones = const.tile([1, 128], F32R)
        nc.vector.memset(ones.bitcast(F32)[:, :], 1.0)
        mrow = const.tile([1, 512], F32R)
        nc.vector.memset(mrow.bitcast(F32)[:, :], MAGIC)
        mrow_n = const.tile([1, 512], F32R)
        nc.vector.memset(mrow_n.bitcast(F32)[:, :], -MAGIC)
        s_run = const.tile([128, NSL], F32)
        nc.vector.memset(s_run[:, :], 0.0)
        sin_warm = const.tile([128, NSL], F16)
        nc.scalar.activation(sin_warm[:, :], s_run[:, :],
                             mybir.ActivationFunctionType.Sin,
                             bias=0.0, scale=NEG2PI)

        # warm the PE p-state during input loads: dummy rank-1 matmuls with
        # no input deps beyond the tiny memset consts
        warm = pu.tile([128, KSH], F32, tag="C", name="warm")
        for _ in range(10):
            nc.tensor.matmul(warm[0:1, 0:512], lhsT=ones[:, 0:1],
                             rhs=mrow[:, :], start=True, stop=True)

        csT = const.tile([128, NSL, N], F16)

        def emit_chunk_front(t, ci, mmc, csb):
            """C psum (4-pass magic), wrap (cos phases), Sin (sin half)."""
            C = pu.tile([128, KSH], F32, tag="C", name=f"C{t}")
            for off in range(0, KSH, 512):
                co = C[:, off:off + 512]
                nc.tensor.matmul(co, lhsT=cts[:, 128 * t:128 * (t + 1)],
                                 rhs=km[:, off:off + 512], start=True, stop=False)
                nc.tensor.matmul(co, lhsT=ones[:, :], rhs=mrow[:, :],
                                 start=False, stop=False)
                nc.tensor.matmul(co, lhsT=ones[:, :], rhs=mrow_n[:, :],
                                 start=False, stop=False)
                nc.tensor.matmul(co, lhsT=cts[:, 128 * t:128 * (t + 1)],
                                 rhs=kmn[:, off:off + 512], start=False, stop=True)
            nc.vector.add_range_wrap(out=mmc[:, ci, :], in_=C[:, :],
                                     shift=-0.25, bound=0.5, period=1.0)
            nc.scalar.activation(csb[:, ci, 0:KSH], C[:, :],
                                 mybir.ActivationFunctionType.Sin,
                                 bias=0.0, scale=NEG2PI)

        def emit_chunk_back(t, ci, csb):
            """S matmuls + transpose for a chunk whose cs is complete."""
            cs = csb[:, ci, :]
            s_ch = psm.tile([128, NSL], F32, tag="S", name=f"sch{t}")
            for s in range(NSL):
                nc.tensor.matmul(s_ch[:, s:s + 1],
                                 lhsT=cs[:, 128 * s:128 * (s + 1)],
                                 rhs=qt16[:, t:t + 1], start=True, stop=True)
            nc.vector.tensor_tensor(out=s_run[:, :], in0=s_run[:, :],
                                    in1=s_ch[:, :], op=mybir.AluOpType.add)
            nc.sync.dma_start_transpose(
                out=csT[:, :, 128 * t:128 * (t + 1)], in_=cs)

        NB = NCH // QB
        prev = None           # (tb, csb) of the previous batch
        for tb in range(0, NCH, QB):
            last = tb == NCH - QB
            mmc = wk.tile([128, QB, KSH], F16, tag="mmc", name=f"mmc{tb}")
            csb = wk.tile([128, QB, 2 * KSH], F16, tag="cs", name=f"cs{tb}")
            for ci in range(QB):
                emit_chunk_front(tb + ci, ci, mmc, csb)
                if last:
                    # sin-half transpose can go before the cos Sin lands
                    nc.sync.dma_start_transpose(
                        out=csT[:, 0:NSL // 2,
                                128 * (tb + ci):128 * (tb + ci + 1)],
                        in_=csb[:, ci, 0:KSH])
                # previous batch's S/transpose interleaved for pipelining
                if prev is not None:
                    emit_chunk_back(prev[0] + ci, ci, prev[1])
            nc.scalar.activation(csb[:, :, KSH:2 * KSH], mmc[:, :, :],
                                 mybir.ActivationFunctionType.Sin,
                                 bias=0.0, scale=NEG2PI)
            prev = (tb, csb)
        for ci in range(QB):
            t = prev[0] + ci
            cs = prev[1][:, ci, :]
            s_ch = psm.tile([128, NSL], F32, tag="S", name=f"sch{t}")
            for s in range(NSL):
                nc.tensor.matmul(s_ch[:, s:s + 1],
                                 lhsT=cs[:, 128 * s:128 * (s + 1)],
                                 rhs=qt16[:, t:t + 1], start=True, stop=True)
            nc.vector.tensor_tensor(out=s_run[:, :], in0=s_run[:, :],
                                    in1=s_ch[:, :], op=mybir.AluOpType.add)
            nc.sync.dma_start_transpose(
                out=csT[:, NSL // 2:NSL, 128 * t:128 * (t + 1)],
                in_=cs[:, KSH:2 * KSH])

        # ---- w = expfac * S (fp16 columns) ----
        w = const.tile([128, NSL], F16)
        nc.vector.tensor_tensor(out=w[:, :], in0=s_run[:, :], in1=ef[:, :],
                                op=mybir.AluOpType.mult)

        # ---- pass 2: recip[128c+p] via 16-deep flipped-matmul psum chains.
        # Store in two parts so the bulk ships while the last chunks (whose
        # transposes land last) finish their chains. ----
        r_acc = pacc.tile([128, NCH], F32, name="r_acc")
        outsb = const.tile([128, NCH], F32)
        SPLIT = NCH - QB * 2
        for c in range(NCH):
            for s in range(NSL):
                nc.tensor.matmul(r_acc[:, c:c + 1],
                                 lhsT=csT[:, s, 128 * c:128 * (c + 1)],
                                 rhs=w[:, s:s + 1],
                                 start=(s == 0), stop=(s == NSL - 1))
            if c == SPLIT - 1:
                nc.vector.tensor_copy(outsb[:, 0:SPLIT], r_acc[:, 0:SPLIT])
                nc.sync.dma_start(out=recp_d[:, 0:SPLIT],
                                  in_=outsb[:, 0:SPLIT])
        nc.vector.tensor_copy(outsb[:, SPLIT:NCH], r_acc[:, SPLIT:NCH])
        nc.sync.dma_start(out=recp_d[:, SPLIT:NCH], in_=outsb[:, SPLIT:NCH])

    nc.compile()
    return nc


def _get_prog():
    if "prog" not in _PROG:
        _PROG["prog"] = _build_program()
    return _PROG["prog"]


def _make_in_maps(coords, q, cell_inv, kvec, expfac):
    in_maps = []
    for c in range(NCORES):
        b, ks = divmod(c, NCORES // B)
        sl = slice(KSH * ks, KSH * (ks + 1))
        ct = np.ascontiguousarray(coords[b].T, dtype=np.float32)
        ef = np.ascontiguousarray(expfac[sl], dtype=np.float32)
        ef8 = ef.reshape(NSL // 2, 128).T          # [128, 8]
        kmT = np.ascontiguousarray(
            (kvec[sl].astype(np.float32) @ cell_inv.astype(np.float32)).T)
        in_maps.append({
            "coordsT": ct,
            "qT": np.ascontiguousarray(q[b].reshape(NCH, 128).T, dtype=np.float32),
            "km2": np.ascontiguousarray(np.concatenate([kmT, -kmT], axis=1)),
            "ef2": np.ascontiguousarray(np.concatenate([ef8, ef8], axis=1)),
        })
    return in_maps


def _finalize(results, q, volume, bewald):
    recip = np.zeros((B, N), np.float32)
    for c in range(NCORES):
        b = c // (NCORES // B)
        recip[b] += results[c]["recp"].T.reshape(-1)
    scale1 = np.float32(BOHR / (math.pi * float(volume[0])))
    scale2 = np.float32(2.0 * float(bewald[0]) * BOHR / math.sqrt(math.pi))
    phi = (recip * scale1 - q.astype(np.float32) * scale2).astype(np.float32)
    e = (np.float32(0.5) * q.astype(np.float32) * phi).astype(np.float32)
    return e, phi


def kernel(coords, q, cell_inv, kvec, expfac, volume, bewald):
    from concourse.bass_utils import run_bass_kernel_spmd

    nc = _get_prog()
    in_maps = _make_in_maps(coords, q, cell_inv, kvec, expfac)
    res = run_bass_kernel_spmd(nc, in_maps, list(range(NCORES))).results
    return _finalize(res, q, volume, bewald)
